# revision 23
# baseline (speedup 1.0000x reference)
"""Trainium2 Bass kernel for nn_DecoderLSTM (ragged LSTM decoder + vocab projection).

Strategy (8 NeuronCores, SPMD):
  - Host: stable-sort batch by descending caption length, gather embeddings for
    the *active* (b, t) pairs only (lengths sorted desc => active set at step t
    is a prefix of the batch), pre-transpose/pack all matmul operands, cast to
    bf16.
  - Device (identical program on all 8 cores; per-core data = lin_W vocab
    shard of 4000 rows):
      * LSTM recurrence over Td steps, full batch replicated on every core.
        gates = [x; h] @ [W_ih; W_hh].T as 8 accumulating K=128 matmuls with
        the (bf16) weights as the moving operand, into one PSUM region.
      * h kept in transposed packed layout HT [128, 4, 64+NA] (bf16) via PE
        transposes, so it directly serves as the stationary operand of both
        the next-step gates matmul and the prediction matmul.
      * Prediction matmul over packed active rows in M=128 chunks (full PE
        stationary utilization), N = 4000 local vocab, interleaved with the
        recurrence as h columns become available.
      * Inactive (b, t) outputs zero-filled by DMA from a zero tile.
  - Host: concatenate the 8 vocab shards.

kernel() accepts the full unsharded inputs and returns
(predictions, caps_sorted, decode_lengths, sort_ind) exactly like reference().
"""

import math

import numpy as np
import ml_dtypes

import concourse.bass as bass
import concourse.mybir as mybir
import concourse.tile as tile
from concourse import bacc
from concourse.bass_utils import run_bass_kernel_spmd
from concourse.masks import make_identity

B, T, E, H, V = 64, 52, 512, 512, 32000
NCORES = 8
VL = V // NCORES          # 4000 local vocab rows per core
TDF = T - 1               # 51 output timesteps
G = 4 * H                 # 2048 gate columns
KE = E // 128             # 4 K-chunks for the x side
KH = H // 128             # 4 K-chunks for the h side
NT = 512                  # matmul moving-operand chunk (one PSUM bank of fp32)
PN = 500                  # prediction matmul N-chunk (4000 = 8 * 500)

F32 = mybir.dt.float32
BF16 = mybir.dt.bfloat16
AF = mybir.ActivationFunctionType

# test harness hooks
TRACE = False
TRACE_KWARGS = {}
LAST_RESULTS = None


def _plan_gx(bs):
    """Group steps into 128-row G_x tiles with each step's rows starting at a
    32-aligned in-tile offset (engine partition accesses must be 32-aligned).
    Returns (gidx[t], gpos[t], rows_per_group)."""
    gidx, gpos, rows = [], [], []
    cur = 0
    for bt in bs:
        bt32 = -(-bt // 32) * 32
        if cur + bt32 > 128 and cur > 0:
            rows.append(cur)
            cur = 0
        gidx.append(len(rows))
        gpos.append(cur)
        cur += bt32
    rows.append(cur)
    return gidx, gpos, rows


def _build(nc, Td, bs, off, na, has_linb):
    """Emit the Tile program. bs[t] = #active batch items at step t (prefixes,
    non-increasing); off[t] = sum(bs[:t]); na = off[Td] total active pairs."""
    gidx, gpos, grows = _plan_gx(bs)
    ngrp = len(grows)
    xt_d = nc.dram_tensor("xt", [128, KE, ngrp * 128], BF16, kind="ExternalInput")
    ht0_d = nc.dram_tensor("ht0", [128, KH, B], BF16, kind="ExternalInput")
    wx_d = nc.dram_tensor("wx", [128, KE, G], BF16, kind="ExternalInput")
    wh_d = nc.dram_tensor("wh", [128, KH, G], BF16, kind="ExternalInput")
    lw_d = nc.dram_tensor("lw", [128, KH, VL], BF16, kind="ExternalInput")
    bias_d = nc.dram_tensor("bias", [128, G], F32, kind="ExternalInput")
    if has_linb:
        linb_d = nc.dram_tensor("linb", [128, VL], F32, kind="ExternalInput")
    # packed active rows only: chunk m -> rows [128m, 128m+mw) contiguous, so
    # every output DMA is one big contiguous descriptor across all 16 SDMA
    # engines; the host scatters rows into the zero-initialized full output
    out_d = nc.dram_tensor("out", [na, VL], F32, kind="ExternalOutput")

    n_chunks = math.ceil(na / 128)

    with tile.TileContext(nc) as tc:
        with (
            tc.tile_pool(name="const", bufs=1) as const,
            tc.tile_pool(name="gpsum", bufs=4, space="PSUM") as gpsum,
            tc.tile_pool(name="ppsum", bufs=2, space="PSUM") as ppsum,
            tc.tile_pool(name="tpsum", bufs=2, space="PSUM") as tpsum,
            tc.tile_pool(name="work", bufs=2) as work,
            tc.tile_pool(name="gxp", bufs=3) as gxp,
            tc.tile_pool(name="pwork", bufs=2) as pwork,
        ):
            WX = const.tile([128, KE, G], BF16, tag="WX")
            nc.sync.dma_start(WX[:], wx_d[:, :, :])
            WH = const.tile([128, KH, G], BF16, tag="WH")
            nc.sync.dma_start(WH[:], wh_d[:, :, :])
            LW = const.tile([128, KH, VL], BF16, tag="LW")
            nc.sync.dma_start(LW[:], lw_d[:, :, :])
            XT = const.tile([128, KE, ngrp * 128], BF16, tag="XT")
            nc.sync.dma_start(XT[:], xt_d[:, :, :])
            BIAS = const.tile([128, G], F32, tag="BIAS")
            nc.sync.dma_start(BIAS[:], bias_d[:, :])
            # h-transposed history: cols 0:B = h0 (image features), block t at
            # B+off[t] (width bs[t])
            HT = const.tile([128, KH, B + na], BF16, tag="HT")
            nc.sync.dma_start(HT[:, :, 0:B], ht0_d[:, :, :])
            if has_linb:
                LB = const.tile([128, VL], F32, tag="LB")
                nc.sync.dma_start(LB[:], linb_d[:, :])
            CST = const.tile([B, H], F32, tag="CST")
            nc.vector.memset(CST[:], 0.0)
            IDN = const.tile([B, B], BF16, tag="IDN")
            make_identity(nc, IDN[:])

            def emit_pred_chunk(m):
                base = m * 128
                mw = min(128, na - base)
                pred = pwork.tile([128, VL], F32, tag="pred")
                for n in range(VL // PN):
                    pp = ppsum.tile([128, PN], F32, tag="pp")
                    for k in range(KH):
                        nc.tensor.matmul(
                            pp[:mw, :],
                            HT[:, k, B + base : B + base + mw],
                            LW[:, k, n * PN : (n + 1) * PN],
                            start=(k == 0),
                            stop=(k == KH - 1),
                        )
                    if has_linb:
                        nc.vector.tensor_add(
                            pred[:mw, n * PN : (n + 1) * PN], pp[:mw, :],
                            LB[:mw, n * PN : (n + 1) * PN],
                        )
                    else:
                        nc.any.tensor_copy(
                            out=pred[:mw, n * PN : (n + 1) * PN], in_=pp[:mw, :]
                        )
                nc.sync.dma_start(out_d[base : base + mw, :], pred[:mw, :])

            # G_x = x @ W_x.T + bias, precomputed per 128-row aligned group
            gx_tiles = {}

            def emit_gx(g):
                rows = grows[g]
                gx = gxp.tile([128, G], F32, tag="gx")
                for n in range(G // NT):
                    ps = gpsum.tile([128, NT], F32, tag="gps")
                    for k in range(KE):
                        nc.tensor.matmul(
                            ps[:rows, :],
                            XT[:, k, g * 128 : g * 128 + rows],
                            WX[:, k, n * NT : (n + 1) * NT],
                            start=(k == 0),
                            stop=(k == KE - 1),
                        )
                    nc.vector.tensor_add(
                        gx[:rows, n * NT : (n + 1) * NT], ps[:rows, :],
                        BIAS[:rows, n * NT : (n + 1) * NT],
                    )
                gx_tiles[g] = gx

            emit_gx(0)
            if ngrp > 1:
                emit_gx(1)

            pred_done = 0
            for t in range(Td):
                bt = bs[t]
                g = gidx[t]
                if t > 0 and g != gidx[t - 1] and g + 1 < ngrp:
                    emit_gx(g + 1)
                p0 = gpos[t]
                gx = gx_tiles[g]
                hcol = 0 if t == 0 else B + int(off[t - 1])
                # gate layout (host-permuted): [g | i | f | o]; per-512-column
                # PSUM tiles so elementwise pipelines with the matmul stream
                acts = []
                for n in range(G // NT):
                    ps = gpsum.tile([B, NT], F32, tag="gps")
                    for k in range(KH):
                        nc.tensor.matmul(
                            ps[:bt, :],
                            HT[:, k, hcol : hcol + bt],
                            WH[:, k, n * NT : (n + 1) * NT],
                            start=(k == 0),
                            stop=(k == KH - 1),
                        )
                    pre = work.tile([B, NT], F32, tag=f"pre{n}")
                    nc.vector.tensor_add(
                        pre[:bt, :], ps[:bt, :],
                        gx[p0 : p0 + bt, n * NT : (n + 1) * NT],
                    )
                    av = work.tile([B, NT], F32, tag=f"act{n}")
                    nc.scalar.activation(
                        av[:bt, :], pre[:bt, :], AF.Tanh if n == 0 else AF.Sigmoid
                    )
                    acts.append(av)
                gv, iv, fv, ov = acts
                ig = work.tile([B, H], F32, tag="ig")
                nc.vector.tensor_mul(ig[:bt, :], iv[:bt, :], gv[:bt, :])
                nc.vector.tensor_mul(CST[:bt, :], fv[:bt, :], CST[:bt, :])
                nc.vector.tensor_add(CST[:bt, :], CST[:bt, :], ig[:bt, :])
                tch = work.tile([B, H], F32, tag="tch")
                nc.scalar.activation(tch[:bt, :], CST[:bt, :], AF.Tanh)
                hbf = work.tile([B, H], BF16, tag="hbf")
                nc.vector.tensor_mul(hbf[:bt, :], ov[:bt, :], tch[:bt, :])
                ocol = B + int(off[t])
                for k in range(KH):
                    tp = tpsum.tile([128, B], BF16, tag="tp")
                    nc.tensor.transpose(
                        tp[:, :bt], hbf[:bt, k * 128 : (k + 1) * 128], IDN[:bt, :bt]
                    )
                    nc.scalar.copy(HT[:, k, ocol : ocol + bt], tp[:, :bt])
                avail = int(off[t]) + bt
                while pred_done < n_chunks and (pred_done + 1) * 128 <= avail:
                    emit_pred_chunk(pred_done)
                    pred_done += 1
            while pred_done < n_chunks:
                emit_pred_chunk(pred_done)
                pred_done += 1
    return nc


def _kpack(m):
    """[D, C] (D % 128 == 0) -> [128, D//128, C] K-chunked layout."""
    d, c = m.shape
    return np.ascontiguousarray(
        m.reshape(d // 128, 128, c).transpose(1, 0, 2)
    )


def kernel(image_features, encoded_captions, caption_lengths,
           emb_W, W_ih, W_hh, b_ih, b_hh, lin_W, lin_b):
    global LAST_RESULTS
    bf = ml_dtypes.bfloat16
    img = np.asarray(image_features, np.float32)
    caps_in = np.asarray(encoded_captions)
    cl = np.asarray(caption_lengths)
    embW = np.asarray(emb_W, np.float32)
    Wih = np.asarray(W_ih, np.float32)
    Whh = np.asarray(W_hh, np.float32)
    bih = np.asarray(b_ih, np.float32)
    bhh = np.asarray(b_hh, np.float32)
    linW = np.asarray(lin_W, np.float32)
    linb = np.asarray(lin_b, np.float32)

    lengths = cl[:, 0]
    sort_ind = np.argsort(-lengths, kind="stable")
    caps = caps_in[sort_ind]
    dl = lengths[sort_ind] - 1
    img_s = img[sort_ind]

    Td = int(dl.max())
    bs = [int((dl > t).sum()) for t in range(Td)]
    off = np.concatenate([[0], np.cumsum(bs)]).astype(np.int64)
    na = int(off[Td])

    gidx, gpos, grows = _plan_gx(bs)
    ngrp = len(grows)
    xp = np.zeros((ngrp * 128, E), np.float32)
    for t in range(Td):
        r0 = gidx[t] * 128 + gpos[t]
        xp[r0:r0 + bs[t]] = embW[caps[: bs[t], t]]

    # gate order [g, i, f, o] (torch rows are i, f, g, o)
    perm = np.concatenate([
        np.arange(2 * H, 3 * H), np.arange(0, H),
        np.arange(H, 2 * H), np.arange(3 * H, 4 * H),
    ])
    xt = _kpack(np.ascontiguousarray(xp.T).astype(bf))
    ht0 = _kpack(np.ascontiguousarray(img_s.T).astype(bf))
    wx = _kpack(np.ascontiguousarray(Wih[perm].T).astype(bf))
    wh = _kpack(np.ascontiguousarray(Whh[perm].T).astype(bf))
    bias = np.ascontiguousarray(
        np.broadcast_to((bih + bhh)[perm].astype(np.float32), (128, G))
    )
    has_linb = bool(np.any(linb != 0.0))

    nc = bacc.Bacc("TRN2", target_bir_lowering=False, debug=False,
                   num_devices=NCORES)
    _build(nc, Td, bs, off, na, has_linb)
    nc.finalize()

    in_maps = []
    for c in range(NCORES):
        lw = _kpack(np.ascontiguousarray(linW[c * VL:(c + 1) * VL].T).astype(bf))
        m = dict(xt=xt, ht0=ht0, wx=wx, wh=wh, lw=lw, bias=bias)
        if has_linb:
            m["linb"] = np.ascontiguousarray(
                np.broadcast_to(linb[c * VL:(c + 1) * VL].astype(np.float32),
                                (128, VL)))
        in_maps.append(m)

    res = run_bass_kernel_spmd(
        nc, in_maps, core_ids=list(range(NCORES)),
        trace=TRACE, trace_cores=list(range(NCORES)) if TRACE else None,
        **TRACE_KWARGS,
    )
    LAST_RESULTS = res

    preds = np.zeros((B, TDF, V), np.float32)
    for c in range(NCORES):
        o = res.results[c]["out"]
        sh = slice(c * VL, (c + 1) * VL)
        for t in range(Td):
            preds[: bs[t], t, sh] = o[off[t]:off[t + 1]]
    return preds, caps, dl, sort_ind


# revision 30
# speedup vs baseline: 1.1364x; 1.1364x over previous
"""Trainium2 Bass kernel for nn_DecoderLSTM (ragged LSTM decoder + vocab projection).

Strategy (8 NeuronCores, SPMD):
  - Host: stable-sort batch by descending caption length, gather embeddings for
    the *active* (b, t) pairs only (lengths sorted desc => active set at step t
    is a prefix of the batch), pre-transpose/pack all matmul operands, cast to
    bf16.
  - Device (identical program on all 8 cores; per-core data = lin_W vocab
    shard of 4000 rows):
      * LSTM recurrence over Td steps, full batch replicated on every core.
        gates = [x; h] @ [W_ih; W_hh].T as 8 accumulating K=128 matmuls with
        the (bf16) weights as the moving operand, into one PSUM region.
      * h kept in transposed packed layout HT [128, 4, 64+NA] (bf16) via PE
        transposes, so it directly serves as the stationary operand of both
        the next-step gates matmul and the prediction matmul.
      * Prediction matmul over packed active rows in M=128 chunks (full PE
        stationary utilization), N = 4000 local vocab, interleaved with the
        recurrence as h columns become available.
      * Inactive (b, t) outputs zero-filled by DMA from a zero tile.
  - Host: concatenate the 8 vocab shards.

kernel() accepts the full unsharded inputs and returns
(predictions, caps_sorted, decode_lengths, sort_ind) exactly like reference().
"""

import math

import numpy as np
import ml_dtypes

import concourse.bass as bass
import concourse.mybir as mybir
import concourse.tile as tile
from concourse import bacc
from concourse.bass_utils import run_bass_kernel_spmd
from concourse.masks import make_identity

B, T, E, H, V = 64, 52, 512, 512, 32000
NCORES = 8
VL = V // NCORES          # 4000 local vocab rows per core
TDF = T - 1               # 51 output timesteps
G = 4 * H                 # 2048 gate columns
KE = E // 128             # 4 K-chunks for the x side
KH = H // 128             # 4 K-chunks for the h side
NT = 512                  # matmul moving-operand chunk (one PSUM bank of fp32)
PN = 500                  # prediction matmul N-chunk (4000 = 8 * 500)

F32 = mybir.dt.float32
BF16 = mybir.dt.bfloat16
AF = mybir.ActivationFunctionType

# test harness hooks
TRACE = False
TRACE_KWARGS = {}
LAST_RESULTS = None


def _plan_gx(bs):
    """Group steps into 128-row G_x tiles with each step's rows starting at a
    32-aligned in-tile offset (engine partition accesses must be 32-aligned).
    Returns (gidx[t], gpos[t], rows_per_group)."""
    gidx, gpos, rows = [], [], []
    cur = 0
    for bt in bs:
        bt32 = -(-bt // 32) * 32
        if cur + bt32 > 128 and cur > 0:
            rows.append(cur)
            cur = 0
        gidx.append(len(rows))
        gpos.append(cur)
        cur += bt32
    rows.append(cur)
    return gidx, gpos, rows


def _build(nc, Td, bs, off, na, has_linb):
    """Emit the Tile program. bs[t] = #active batch items at step t (prefixes,
    non-increasing); off[t] = sum(bs[:t]); na = off[Td] total active pairs."""
    gidx, gpos, grows = _plan_gx(bs)
    ngrp = len(grows)
    xt_d = nc.dram_tensor("xt", [128, KE, ngrp * 128], BF16, kind="ExternalInput")
    ht0_d = nc.dram_tensor("ht0", [128, KH, B], BF16, kind="ExternalInput")
    wx_d = nc.dram_tensor("wx", [128, KE, G], BF16, kind="ExternalInput")
    wh_d = nc.dram_tensor("wh", [128, KH, G], BF16, kind="ExternalInput")
    lw_d = nc.dram_tensor("lw", [128, KH, VL], BF16, kind="ExternalInput")
    bias_d = nc.dram_tensor("bias", [128, G], F32, kind="ExternalInput")
    if has_linb:
        linb_d = nc.dram_tensor("linb", [128, VL], F32, kind="ExternalInput")
    # packed active rows only: chunk m -> rows [128m, 128m+mw) contiguous, so
    # every output DMA is one big contiguous descriptor across all 16 SDMA
    # engines; the host scatters rows into the zero-initialized full output
    out_d = nc.dram_tensor("out", [na, VL], F32, kind="ExternalOutput")

    n_chunks = math.ceil(na / 128)

    with tile.TileContext(nc) as tc:
        with (
            tc.tile_pool(name="const", bufs=1) as const,
            tc.tile_pool(name="gpsum", bufs=4, space="PSUM") as gpsum,
            tc.tile_pool(name="ppsum", bufs=2, space="PSUM") as ppsum,
            tc.tile_pool(name="tpsum", bufs=2, space="PSUM") as tpsum,
            tc.tile_pool(name="work", bufs=2) as work,
            tc.tile_pool(name="gxp", bufs=3) as gxp,
            tc.tile_pool(name="pwork", bufs=2) as pwork,
        ):
            WX = const.tile([128, KE, G], BF16, tag="WX")
            nc.sync.dma_start(WX[:], wx_d[:, :, :])
            WH = const.tile([128, KH, G], BF16, tag="WH")
            nc.sync.dma_start(WH[:], wh_d[:, :, :])
            LW = const.tile([128, KH, VL], BF16, tag="LW")
            nc.sync.dma_start(LW[:], lw_d[:, :, :])
            XT = const.tile([128, KE, ngrp * 128], BF16, tag="XT")
            nc.sync.dma_start(XT[:], xt_d[:, :, :])
            BIAS = const.tile([128, G], F32, tag="BIAS")
            nc.sync.dma_start(BIAS[:], bias_d[:, :])
            # h-transposed history: cols 0:B = h0 (image features), block t at
            # B+off[t] (width bs[t])
            HT = const.tile([128, KH, B + na], BF16, tag="HT")
            nc.sync.dma_start(HT[:, :, 0:B], ht0_d[:, :, :])
            if has_linb:
                LB = const.tile([128, VL], F32, tag="LB")
                nc.sync.dma_start(LB[:], linb_d[:, :])
            CST = const.tile([B, H], F32, tag="CST")
            nc.vector.memset(CST[:], 0.0)
            IDN = const.tile([B, B], BF16, tag="IDN")
            make_identity(nc, IDN[:])

            # prediction work is emitted in (chunk, n-slice) units interleaved
            # between recurrence steps: PE executes its stream in order, so
            # these units fill PE during the per-step elementwise chain
            pred_state = {}  # m -> pred sbuf tile

            def emit_pred_unit(m, n):
                base = m * 128
                mw = min(128, na - base)
                if n == 0:
                    pred_state[m] = pwork.tile([128, VL], F32, tag="pred",
                                               name=f"pred{m}")
                pred = pred_state[m]
                pp = ppsum.tile([128, PN], F32, tag="pp")
                for k in range(KH):
                    nc.tensor.matmul(
                        pp[:mw, :],
                        HT[:, k, B + base : B + base + mw],
                        LW[:, k, n * PN : (n + 1) * PN],
                        start=(k == 0),
                        stop=(k == KH - 1),
                    )
                if has_linb:
                    nc.vector.tensor_add(
                        pred[:mw, n * PN : (n + 1) * PN], pp[:mw, :],
                        LB[:mw, n * PN : (n + 1) * PN],
                    )
                elif n % 2 == 0:
                    nc.vector.tensor_copy(pred[:mw, n * PN : (n + 1) * PN],
                                          pp[:mw, :])
                else:
                    nc.scalar.copy(pred[:mw, n * PN : (n + 1) * PN], pp[:mw, :])
                if n == VL // PN - 1:
                    nc.sync.dma_start(out_d[base : base + mw, :], pred[:mw, :])
                    del pred_state[m]

            # G_x = x @ W_x.T + bias, precomputed per 128-row aligned group
            gx_tiles = {}

            def emit_gx(g):
                rows = grows[g]
                gx = gxp.tile([128, G], F32, tag="gx")
                for n in range(G // NT):
                    ps = gpsum.tile([128, NT], F32, tag="gps")
                    for k in range(KE):
                        nc.tensor.matmul(
                            ps[:rows, :],
                            XT[:, k, g * 128 : g * 128 + rows],
                            WX[:, k, n * NT : (n + 1) * NT],
                            start=(k == 0),
                            stop=(k == KE - 1),
                        )
                    nc.vector.tensor_add(
                        gx[:rows, n * NT : (n + 1) * NT], ps[:rows, :],
                        BIAS[:rows, n * NT : (n + 1) * NT],
                    )
                gx_tiles[g] = gx

            emit_gx(0)
            if ngrp > 1:
                emit_gx(1)

            from collections import deque
            pred_units = deque()
            pred_done = 0
            for t in range(Td):
                bt = bs[t]
                g = gidx[t]
                if t > 0 and g != gidx[t - 1] and g + 1 < ngrp:
                    emit_gx(g + 1)
                p0 = gpos[t]
                gx = gx_tiles[g]
                hcol = 0 if t == 0 else B + int(off[t - 1])
                # gate layout (host-permuted): [g | i | f | o]; per-512-column
                # PSUM tiles so elementwise pipelines with the matmul stream
                acts = []
                for n in range(G // NT):
                    ps = gpsum.tile([B, NT], F32, tag="gps")
                    for k in range(KH):
                        nc.tensor.matmul(
                            ps[:bt, :],
                            HT[:, k, hcol : hcol + bt],
                            WH[:, k, n * NT : (n + 1) * NT],
                            start=(k == 0),
                            stop=(k == KH - 1),
                        )
                    pre = work.tile([B, NT], F32, tag=f"pre{n}")
                    nc.vector.tensor_add(
                        pre[:bt, :], ps[:bt, :],
                        gx[p0 : p0 + bt, n * NT : (n + 1) * NT],
                    )
                    av = work.tile([B, NT], F32, tag=f"act{n}")
                    nc.scalar.activation(
                        av[:bt, :], pre[:bt, :], AF.Tanh if n == 0 else AF.Sigmoid
                    )
                    acts.append(av)
                gv, iv, fv, ov = acts
                ig = work.tile([B, H], F32, tag="ig")
                nc.gpsimd.tensor_mul(ig[:bt, :], iv[:bt, :], gv[:bt, :])
                nc.vector.tensor_mul(CST[:bt, :], fv[:bt, :], CST[:bt, :])
                nc.vector.tensor_add(CST[:bt, :], CST[:bt, :], ig[:bt, :])
                tch = work.tile([B, H], F32, tag="tch")
                nc.scalar.activation(tch[:bt, :], CST[:bt, :], AF.Tanh)
                hbf = work.tile([B, H], BF16, tag="hbf")
                nc.vector.tensor_mul(hbf[:bt, :], ov[:bt, :], tch[:bt, :])
                ocol = B + int(off[t])
                for k in range(KH):
                    tp = tpsum.tile([128, B], BF16, tag="tp")
                    nc.tensor.transpose(
                        tp[:, :bt], hbf[:bt, k * 128 : (k + 1) * 128], IDN[:bt, :bt]
                    )
                    nc.scalar.copy(HT[:, k, ocol : ocol + bt], tp[:, :bt])
                avail = int(off[t]) + bt
                while pred_done < n_chunks and (pred_done + 1) * 128 <= avail:
                    for n in range(VL // PN):
                        pred_units.append((pred_done, n))
                    pred_done += 1
                # spread pred work between steps to keep PE fed during the
                # per-step elementwise chain
                budget = 3 if t < Td - 1 else len(pred_units)
                for _ in range(min(budget, len(pred_units))):
                    emit_pred_unit(*pred_units.popleft())
            while pred_done < n_chunks:
                for n in range(VL // PN):
                    pred_units.append((pred_done, n))
                pred_done += 1
            while pred_units:
                emit_pred_unit(*pred_units.popleft())
    return nc


def _kpack(m):
    """[D, C] (D % 128 == 0) -> [128, D//128, C] K-chunked layout."""
    d, c = m.shape
    return np.ascontiguousarray(
        m.reshape(d // 128, 128, c).transpose(1, 0, 2)
    )


def kernel(image_features, encoded_captions, caption_lengths,
           emb_W, W_ih, W_hh, b_ih, b_hh, lin_W, lin_b):
    global LAST_RESULTS
    bf = ml_dtypes.bfloat16
    img = np.asarray(image_features, np.float32)
    caps_in = np.asarray(encoded_captions)
    cl = np.asarray(caption_lengths)
    embW = np.asarray(emb_W, np.float32)
    Wih = np.asarray(W_ih, np.float32)
    Whh = np.asarray(W_hh, np.float32)
    bih = np.asarray(b_ih, np.float32)
    bhh = np.asarray(b_hh, np.float32)
    linW = np.asarray(lin_W, np.float32)
    linb = np.asarray(lin_b, np.float32)

    lengths = cl[:, 0]
    sort_ind = np.argsort(-lengths, kind="stable")
    caps = caps_in[sort_ind]
    dl = lengths[sort_ind] - 1
    img_s = img[sort_ind]

    Td = int(dl.max())
    bs = [int((dl > t).sum()) for t in range(Td)]
    off = np.concatenate([[0], np.cumsum(bs)]).astype(np.int64)
    na = int(off[Td])

    gidx, gpos, grows = _plan_gx(bs)
    ngrp = len(grows)
    xp = np.zeros((ngrp * 128, E), np.float32)
    for t in range(Td):
        r0 = gidx[t] * 128 + gpos[t]
        xp[r0:r0 + bs[t]] = embW[caps[: bs[t], t]]

    # gate order [g, i, f, o] (torch rows are i, f, g, o)
    perm = np.concatenate([
        np.arange(2 * H, 3 * H), np.arange(0, H),
        np.arange(H, 2 * H), np.arange(3 * H, 4 * H),
    ])
    xt = _kpack(np.ascontiguousarray(xp.T).astype(bf))
    ht0 = _kpack(np.ascontiguousarray(img_s.T).astype(bf))
    wx = _kpack(np.ascontiguousarray(Wih[perm].T).astype(bf))
    wh = _kpack(np.ascontiguousarray(Whh[perm].T).astype(bf))
    bias = np.ascontiguousarray(
        np.broadcast_to((bih + bhh)[perm].astype(np.float32), (128, G))
    )
    has_linb = bool(np.any(linb != 0.0))

    nc = bacc.Bacc("TRN2", target_bir_lowering=False, debug=False,
                   num_devices=NCORES)
    _build(nc, Td, bs, off, na, has_linb)
    nc.finalize()

    in_maps = []
    for c in range(NCORES):
        lw = _kpack(np.ascontiguousarray(linW[c * VL:(c + 1) * VL].T).astype(bf))
        m = dict(xt=xt, ht0=ht0, wx=wx, wh=wh, lw=lw, bias=bias)
        if has_linb:
            m["linb"] = np.ascontiguousarray(
                np.broadcast_to(linb[c * VL:(c + 1) * VL].astype(np.float32),
                                (128, VL)))
        in_maps.append(m)

    res = run_bass_kernel_spmd(
        nc, in_maps, core_ids=list(range(NCORES)),
        trace=TRACE, trace_cores=list(range(NCORES)) if TRACE else None,
        **TRACE_KWARGS,
    )
    LAST_RESULTS = res

    preds = np.zeros((B, TDF, V), np.float32)
    for c in range(NCORES):
        o = res.results[c]["out"]
        sh = slice(c * VL, (c + 1) * VL)
        for t in range(Td):
            preds[: bs[t], t, sh] = o[off[t]:off[t + 1]]
    return preds, caps, dl, sort_ind


# revision 33
# speedup vs baseline: 1.1594x; 1.0202x over previous
"""Trainium2 Bass kernel for nn_DecoderLSTM (ragged LSTM decoder + vocab projection).

Strategy (8 NeuronCores, SPMD):
  - Host: stable-sort batch by descending caption length, gather embeddings for
    the *active* (b, t) pairs only (lengths sorted desc => active set at step t
    is a prefix of the batch), pre-transpose/pack all matmul operands, cast to
    bf16.
  - Device (identical program on all 8 cores; per-core data = lin_W vocab
    shard of 4000 rows):
      * LSTM recurrence over Td steps, full batch replicated on every core.
        gates = [x; h] @ [W_ih; W_hh].T as 8 accumulating K=128 matmuls with
        the (bf16) weights as the moving operand, into one PSUM region.
      * h kept in transposed packed layout HT [128, 4, 64+NA] (bf16) via PE
        transposes, so it directly serves as the stationary operand of both
        the next-step gates matmul and the prediction matmul.
      * Prediction matmul over packed active rows in M=128 chunks (full PE
        stationary utilization), N = 4000 local vocab, interleaved with the
        recurrence as h columns become available.
      * Inactive (b, t) outputs zero-filled by DMA from a zero tile.
  - Host: concatenate the 8 vocab shards.

kernel() accepts the full unsharded inputs and returns
(predictions, caps_sorted, decode_lengths, sort_ind) exactly like reference().
"""

import math

import numpy as np
import ml_dtypes

import concourse.bass as bass
import concourse.mybir as mybir
import concourse.tile as tile
from concourse import bacc
from concourse.bass_utils import run_bass_kernel_spmd
from concourse.masks import make_identity

B, T, E, H, V = 64, 52, 512, 512, 32000
NCORES = 8
VL = V // NCORES          # 4000 local vocab rows per core
TDF = T - 1               # 51 output timesteps
G = 4 * H                 # 2048 gate columns
KE = E // 128             # 4 K-chunks for the x side
KH = H // 128             # 4 K-chunks for the h side
NT = 512                  # matmul moving-operand chunk (one PSUM bank of fp32)
PN = 500                  # prediction matmul N-chunk (4000 = 8 * 500)

F32 = mybir.dt.float32
BF16 = mybir.dt.bfloat16
AF = mybir.ActivationFunctionType

# test harness hooks
TRACE = False
TRACE_KWARGS = {}
LAST_RESULTS = None


def _plan_gx(bs):
    """Group steps into 128-row G_x tiles with each step's rows starting at a
    32-aligned in-tile offset (engine partition accesses must be 32-aligned).
    Returns (gidx[t], gpos[t], rows_per_group)."""
    gidx, gpos, rows = [], [], []
    cur = 0
    for bt in bs:
        bt32 = -(-bt // 32) * 32
        if cur + bt32 > 128 and cur > 0:
            rows.append(cur)
            cur = 0
        gidx.append(len(rows))
        gpos.append(cur)
        cur += bt32
    rows.append(cur)
    return gidx, gpos, rows


def _build(nc, Td, bs, off, na, has_linb):
    """Emit the Tile program. bs[t] = #active batch items at step t (prefixes,
    non-increasing); off[t] = sum(bs[:t]); na = off[Td] total active pairs."""
    gidx, gpos, grows = _plan_gx(bs)
    ngrp = len(grows)
    xt_d = nc.dram_tensor("xt", [128, KE, ngrp * 128], BF16, kind="ExternalInput")
    ht0_d = nc.dram_tensor("ht0", [128, KH, B], BF16, kind="ExternalInput")
    wx_d = nc.dram_tensor("wx", [128, KE, G], BF16, kind="ExternalInput")
    wh_d = nc.dram_tensor("wh", [128, KH, G], BF16, kind="ExternalInput")
    lw_d = nc.dram_tensor("lw", [128, KH, VL], BF16, kind="ExternalInput")
    bias_d = nc.dram_tensor("bias", [128, G], F32, kind="ExternalInput")
    if has_linb:
        linb_d = nc.dram_tensor("linb", [128, VL], F32, kind="ExternalInput")
    # packed active rows only: chunk m -> rows [128m, 128m+mw) contiguous, so
    # every output DMA is one big contiguous descriptor across all 16 SDMA
    # engines; the host scatters rows into the zero-initialized full output
    out_d = nc.dram_tensor("out", [na, VL], F32, kind="ExternalOutput")

    n_chunks = math.ceil(na / 128)

    with tile.TileContext(nc) as tc:
        with (
            tc.tile_pool(name="const", bufs=1) as const,
            tc.tile_pool(name="gpsum", bufs=4, space="PSUM") as gpsum,
            tc.tile_pool(name="ppsum", bufs=2, space="PSUM") as ppsum,
            tc.tile_pool(name="tpsum", bufs=2, space="PSUM") as tpsum,
            tc.tile_pool(name="work", bufs=2) as work,
            tc.tile_pool(name="gxp", bufs=3) as gxp,
            tc.tile_pool(name="pwork", bufs=2) as pwork,
        ):
            # load order: what the first G_x groups and step 0 need comes first
            XT = const.tile([128, KE, ngrp * 128], BF16, tag="XT")
            nc.sync.dma_start(XT[:], xt_d[:, :, :])
            WX = const.tile([128, KE, G], BF16, tag="WX")
            nc.sync.dma_start(WX[:], wx_d[:, :, :])
            BIAS = const.tile([128, G], F32, tag="BIAS")
            nc.sync.dma_start(BIAS[:], bias_d[:, :])
            # h-transposed history: cols 0:B = h0 (image features), block t at
            # B+off[t] (width bs[t])
            HT = const.tile([128, KH, B + na], BF16, tag="HT")
            nc.sync.dma_start(HT[:, :, 0:B], ht0_d[:, :, :])
            WH = const.tile([128, KH, G], BF16, tag="WH")
            nc.sync.dma_start(WH[:], wh_d[:, :, :])
            LW = const.tile([128, KH, VL], BF16, tag="LW")
            nc.sync.dma_start(LW[:], lw_d[:, :, :])
            if has_linb:
                LB = const.tile([128, VL], F32, tag="LB")
                nc.sync.dma_start(LB[:], linb_d[:, :])
            CST = const.tile([B, H], F32, tag="CST")
            nc.vector.memset(CST[:], 0.0)
            IDN = const.tile([B, B], BF16, tag="IDN")
            make_identity(nc, IDN[:])

            # prediction work is emitted in (chunk, n-slice) units interleaved
            # between recurrence steps: PE executes its stream in order, so
            # these units fill PE during the per-step elementwise chain
            pred_state = {}  # m -> pred sbuf tile

            def emit_pred_unit(m, n):
                base = m * 128
                mw = min(128, na - base)
                if n == 0:
                    pred_state[m] = pwork.tile([128, VL], F32, tag="pred",
                                               name=f"pred{m}")
                pred = pred_state[m]
                pp = ppsum.tile([128, PN], F32, tag="pp")
                for k in range(KH):
                    nc.tensor.matmul(
                        pp[:mw, :],
                        HT[:, k, B + base : B + base + mw],
                        LW[:, k, n * PN : (n + 1) * PN],
                        start=(k == 0),
                        stop=(k == KH - 1),
                    )
                if has_linb:
                    nc.vector.tensor_add(
                        pred[:mw, n * PN : (n + 1) * PN], pp[:mw, :],
                        LB[:mw, n * PN : (n + 1) * PN],
                    )
                else:
                    nc.scalar.copy(pred[:mw, n * PN : (n + 1) * PN], pp[:mw, :])
                if n == VL // PN - 1:
                    nc.sync.dma_start(out_d[base : base + mw, :], pred[:mw, :])
                    del pred_state[m]

            # G_x = x @ W_x.T + bias, precomputed per 128-row aligned group
            gx_tiles = {}

            def emit_gx(g):
                rows = grows[g]
                gx = gxp.tile([128, G], F32, tag="gx")
                for n in range(G // NT):
                    ps = gpsum.tile([128, NT], F32, tag="gps")
                    for k in range(KE):
                        nc.tensor.matmul(
                            ps[:rows, :],
                            XT[:, k, g * 128 : g * 128 + rows],
                            WX[:, k, n * NT : (n + 1) * NT],
                            start=(k == 0),
                            stop=(k == KE - 1),
                        )
                    nc.vector.tensor_add(
                        gx[:rows, n * NT : (n + 1) * NT], ps[:rows, :],
                        BIAS[:rows, n * NT : (n + 1) * NT],
                    )
                gx_tiles[g] = gx

            emit_gx(0)
            if ngrp > 1:
                emit_gx(1)

            from collections import deque
            pred_units = deque()
            pred_done = 0
            for t in range(Td):
                bt = bs[t]
                g = gidx[t]
                if t > 0 and g != gidx[t - 1] and g + 1 < ngrp:
                    emit_gx(g + 1)
                p0 = gpos[t]
                gx = gx_tiles[g]
                hcol = 0 if t == 0 else B + int(off[t - 1])
                # gate layout (host-permuted): [g | i | f | o]; per-512-column
                # PSUM tiles so elementwise pipelines with the matmul stream
                acts = []
                for n in range(G // NT):
                    ps = gpsum.tile([B, NT], F32, tag="gps")
                    for k in range(KH):
                        nc.tensor.matmul(
                            ps[:bt, :],
                            HT[:, k, hcol : hcol + bt],
                            WH[:, k, n * NT : (n + 1) * NT],
                            start=(k == 0),
                            stop=(k == KH - 1),
                        )
                    pre = work.tile([B, NT], F32, tag=f"pre{n}")
                    nc.vector.tensor_add(
                        pre[:bt, :], ps[:bt, :],
                        gx[p0 : p0 + bt, n * NT : (n + 1) * NT],
                    )
                    av = work.tile([B, NT], F32, tag=f"act{n}")
                    nc.scalar.activation(
                        av[:bt, :], pre[:bt, :], AF.Tanh if n == 0 else AF.Sigmoid
                    )
                    acts.append(av)
                gv, iv, fv, ov = acts
                ig = work.tile([B, H], F32, tag="ig")
                nc.vector.tensor_mul(ig[:bt, :], iv[:bt, :], gv[:bt, :])
                nc.vector.tensor_mul(CST[:bt, :], fv[:bt, :], CST[:bt, :])
                nc.vector.tensor_add(CST[:bt, :], CST[:bt, :], ig[:bt, :])
                tch = work.tile([B, H], F32, tag="tch")
                nc.scalar.activation(tch[:bt, :], CST[:bt, :], AF.Tanh)
                hbf = work.tile([B, H], BF16, tag="hbf")
                nc.vector.tensor_mul(hbf[:bt, :], ov[:bt, :], tch[:bt, :])
                ocol = B + int(off[t])
                for k in range(KH):
                    tp = tpsum.tile([128, B], BF16, tag="tp")
                    nc.tensor.transpose(
                        tp[:, :bt], hbf[:bt, k * 128 : (k + 1) * 128], IDN[:bt, :bt]
                    )
                    nc.scalar.copy(HT[:, k, ocol : ocol + bt], tp[:, :bt])
                avail = int(off[t]) + bt
                while pred_done < n_chunks and (pred_done + 1) * 128 <= avail:
                    for n in range(VL // PN):
                        pred_units.append((pred_done, n))
                    pred_done += 1
                # spread pred work between steps to keep PE fed during the
                # per-step elementwise chain
                budget = 3 if t < Td - 1 else len(pred_units)
                for _ in range(min(budget, len(pred_units))):
                    emit_pred_unit(*pred_units.popleft())
            while pred_done < n_chunks:
                for n in range(VL // PN):
                    pred_units.append((pred_done, n))
                pred_done += 1
            while pred_units:
                emit_pred_unit(*pred_units.popleft())
    return nc


def _kpack(m):
    """[D, C] (D % 128 == 0) -> [128, D//128, C] K-chunked layout."""
    d, c = m.shape
    return np.ascontiguousarray(
        m.reshape(d // 128, 128, c).transpose(1, 0, 2)
    )


def kernel(image_features, encoded_captions, caption_lengths,
           emb_W, W_ih, W_hh, b_ih, b_hh, lin_W, lin_b):
    global LAST_RESULTS
    bf = ml_dtypes.bfloat16
    img = np.asarray(image_features, np.float32)
    caps_in = np.asarray(encoded_captions)
    cl = np.asarray(caption_lengths)
    embW = np.asarray(emb_W, np.float32)
    Wih = np.asarray(W_ih, np.float32)
    Whh = np.asarray(W_hh, np.float32)
    bih = np.asarray(b_ih, np.float32)
    bhh = np.asarray(b_hh, np.float32)
    linW = np.asarray(lin_W, np.float32)
    linb = np.asarray(lin_b, np.float32)

    lengths = cl[:, 0]
    sort_ind = np.argsort(-lengths, kind="stable")
    caps = caps_in[sort_ind]
    dl = lengths[sort_ind] - 1
    img_s = img[sort_ind]

    Td = int(dl.max())
    bs = [int((dl > t).sum()) for t in range(Td)]
    off = np.concatenate([[0], np.cumsum(bs)]).astype(np.int64)
    na = int(off[Td])

    gidx, gpos, grows = _plan_gx(bs)
    ngrp = len(grows)
    xp = np.zeros((ngrp * 128, E), np.float32)
    for t in range(Td):
        r0 = gidx[t] * 128 + gpos[t]
        xp[r0:r0 + bs[t]] = embW[caps[: bs[t], t]]

    # gate order [g, i, f, o] (torch rows are i, f, g, o)
    perm = np.concatenate([
        np.arange(2 * H, 3 * H), np.arange(0, H),
        np.arange(H, 2 * H), np.arange(3 * H, 4 * H),
    ])
    xt = _kpack(np.ascontiguousarray(xp.T).astype(bf))
    ht0 = _kpack(np.ascontiguousarray(img_s.T).astype(bf))
    wx = _kpack(np.ascontiguousarray(Wih[perm].T).astype(bf))
    wh = _kpack(np.ascontiguousarray(Whh[perm].T).astype(bf))
    bias = np.ascontiguousarray(
        np.broadcast_to((bih + bhh)[perm].astype(np.float32), (128, G))
    )
    has_linb = bool(np.any(linb != 0.0))

    nc = bacc.Bacc("TRN2", target_bir_lowering=False, debug=False,
                   num_devices=NCORES)
    _build(nc, Td, bs, off, na, has_linb)
    nc.finalize()

    in_maps = []
    for c in range(NCORES):
        lw = _kpack(np.ascontiguousarray(linW[c * VL:(c + 1) * VL].T).astype(bf))
        m = dict(xt=xt, ht0=ht0, wx=wx, wh=wh, lw=lw, bias=bias)
        if has_linb:
            m["linb"] = np.ascontiguousarray(
                np.broadcast_to(linb[c * VL:(c + 1) * VL].astype(np.float32),
                                (128, VL)))
        in_maps.append(m)

    res = run_bass_kernel_spmd(
        nc, in_maps, core_ids=list(range(NCORES)),
        trace=TRACE, trace_cores=list(range(NCORES)) if TRACE else None,
        **TRACE_KWARGS,
    )
    LAST_RESULTS = res

    preds = np.zeros((B, TDF, V), np.float32)
    for c in range(NCORES):
        o = res.results[c]["out"]
        sh = slice(c * VL, (c + 1) * VL)
        for t in range(Td):
            preds[: bs[t], t, sh] = o[off[t]:off[t + 1]]
    return preds, caps, dl, sort_ind


# revision 38
# speedup vs baseline: 1.1620x; 1.0022x over previous
"""Trainium2 Bass kernel for nn_DecoderLSTM (ragged LSTM decoder + vocab projection).

Strategy (8 NeuronCores, SPMD):
  - Host: stable-sort batch by descending caption length, gather embeddings for
    the *active* (b, t) pairs only (lengths sorted desc => active set at step t
    is a prefix of the batch), pre-transpose/pack all matmul operands, cast to
    bf16.
  - Device (identical program on all 8 cores; per-core data = lin_W vocab
    shard of 4000 rows):
      * LSTM recurrence over Td steps, full batch replicated on every core.
        gates = [x; h] @ [W_ih; W_hh].T as 8 accumulating K=128 matmuls with
        the (bf16) weights as the moving operand, into one PSUM region.
      * h kept in transposed packed layout HT [128, 4, 64+NA] (bf16) via PE
        transposes, so it directly serves as the stationary operand of both
        the next-step gates matmul and the prediction matmul.
      * Prediction matmul over packed active rows in M=128 chunks (full PE
        stationary utilization), N = 4000 local vocab, interleaved with the
        recurrence as h columns become available.
      * Inactive (b, t) outputs zero-filled by DMA from a zero tile.
  - Host: concatenate the 8 vocab shards.

kernel() accepts the full unsharded inputs and returns
(predictions, caps_sorted, decode_lengths, sort_ind) exactly like reference().
"""

import math

import numpy as np
import ml_dtypes

import concourse.bass as bass
import concourse.mybir as mybir
import concourse.tile as tile
from concourse import bacc
from concourse.bass_utils import run_bass_kernel_spmd
from concourse.masks import make_identity

B, T, E, H, V = 64, 52, 512, 512, 32000
NCORES = 8
VL = V // NCORES          # 4000 local vocab rows per core
TDF = T - 1               # 51 output timesteps
G = 4 * H                 # 2048 gate columns
KE = E // 128             # 4 K-chunks for the x side
KH = H // 128             # 4 K-chunks for the h side
NT = 512                  # matmul moving-operand chunk (one PSUM bank of fp32)
PN = 500                  # prediction matmul N-chunk (4000 = 8 * 500)

F32 = mybir.dt.float32
BF16 = mybir.dt.bfloat16
AF = mybir.ActivationFunctionType

# test harness hooks
TRACE = False
TRACE_KWARGS = {}
LAST_RESULTS = None


def _plan_gx(bs):
    """Group steps into 128-row G_x tiles with each step's rows starting at a
    32-aligned in-tile offset (engine partition accesses must be 32-aligned).
    Returns (gidx[t], gpos[t], rows_per_group)."""
    gidx, gpos, rows = [], [], []
    cur = 0
    for bt in bs:
        bt32 = -(-bt // 32) * 32
        if cur + bt32 > 128 and cur > 0:
            rows.append(cur)
            cur = 0
        gidx.append(len(rows))
        gpos.append(cur)
        cur += bt32
    rows.append(cur)
    return gidx, gpos, rows


def _build(nc, Td, bs, off, na, has_linb):
    """Emit the Tile program. bs[t] = #active batch items at step t (prefixes,
    non-increasing); off[t] = sum(bs[:t]); na = off[Td] total active pairs."""
    gidx, gpos, grows = _plan_gx(bs)
    ngrp = len(grows)
    xt_d = nc.dram_tensor("xt", [128, KE, ngrp * 128], BF16, kind="ExternalInput")
    ht0_d = nc.dram_tensor("ht0", [128, KH, B], BF16, kind="ExternalInput")
    wx_d = nc.dram_tensor("wx", [128, KE, G], BF16, kind="ExternalInput")
    wh_d = nc.dram_tensor("wh", [128, KH, G], BF16, kind="ExternalInput")
    lw_d = nc.dram_tensor("lw", [128, KH, VL], BF16, kind="ExternalInput")
    bias_d = nc.dram_tensor("bias", [128, G], F32, kind="ExternalInput")
    if has_linb:
        linb_d = nc.dram_tensor("linb", [128, VL], F32, kind="ExternalInput")
    # packed active rows only: chunk m -> rows [128m, 128m+mw) contiguous, so
    # every output DMA is one big contiguous descriptor across all 16 SDMA
    # engines; the host scatters rows into the zero-initialized full output
    out_d = nc.dram_tensor("out", [na, VL], F32, kind="ExternalOutput")

    n_chunks = math.ceil(na / 128)

    with tile.TileContext(nc) as tc:
        with (
            tc.tile_pool(name="const", bufs=1) as const,
            tc.tile_pool(name="gpsum", bufs=4, space="PSUM") as gpsum,
            tc.tile_pool(name="ppsum", bufs=2, space="PSUM") as ppsum,
            tc.tile_pool(name="tpsum", bufs=2, space="PSUM") as tpsum,
            tc.tile_pool(name="work", bufs=2) as work,
            tc.tile_pool(name="gxp", bufs=3) as gxp,
            tc.tile_pool(name="pwork", bufs=2) as pwork,
        ):
            # load order: what the first G_x groups and step 0 need comes first
            XT = const.tile([128, KE, ngrp * 128], BF16, tag="XT")
            nc.sync.dma_start(XT[:], xt_d[:, :, :])
            WX = const.tile([128, KE, G], BF16, tag="WX")
            nc.sync.dma_start(WX[:], wx_d[:, :, :])
            BIAS = const.tile([128, G], F32, tag="BIAS")
            nc.sync.dma_start(BIAS[:], bias_d[:, :])
            # h-transposed history: cols 0:B = h0 (image features), block t at
            # B+off[t] (width bs[t]); +128 pad cols so stationary slices can
            # always take 128 columns (FWL needs NumWeights==128)
            HT = const.tile([128, KH, B + na + 128], BF16, tag="HT")
            nc.vector.memset(HT[:], 0.0)
            nc.sync.dma_start(HT[:, :, 0:B], ht0_d[:, :, :])
            WH = const.tile([128, KH, G], BF16, tag="WH")
            nc.sync.dma_start(WH[:], wh_d[:, :, :])
            LW = const.tile([128, KH, VL], BF16, tag="LW")
            nc.sync.dma_start(LW[:], lw_d[:, :, :])
            if has_linb:
                LB = const.tile([128, VL], F32, tag="LB")
                nc.sync.dma_start(LB[:], linb_d[:, :])
            CST = const.tile([B, H], F32, tag="CST")
            nc.vector.memset(CST[:], 0.0)
            IDN = const.tile([B, B], BF16, tag="IDN")
            make_identity(nc, IDN[:])

            # prediction work is emitted in (chunk, n-slice) units interleaved
            # between recurrence steps: PE executes its stream in order, so
            # these units fill PE during the per-step elementwise chain
            pred_state = {}  # m -> pred sbuf tile

            def emit_pred_unit(m, n):
                base = m * 128
                mw = min(128, na - base)
                if n == 0:
                    pred_state[m] = pwork.tile([128, VL], F32, tag="pred",
                                               name=f"pred{m}")
                pred = pred_state[m]
                pp = ppsum.tile([128, PN], F32, tag="pp")
                for k in range(KH):
                    nc.tensor.matmul(
                        pp[:, :],
                        HT[:, k, B + base : B + base + 128],
                        LW[:, k, n * PN : (n + 1) * PN],
                        start=(k == 0),
                        stop=(k == KH - 1),
                    )
                if has_linb:
                    nc.vector.tensor_add(
                        pred[:mw, n * PN : (n + 1) * PN], pp[:mw, :],
                        LB[:mw, n * PN : (n + 1) * PN],
                    )
                else:
                    nc.scalar.copy(pred[:mw, n * PN : (n + 1) * PN], pp[:mw, :])
                if n == VL // PN - 1:
                    nc.sync.dma_start(out_d[base : base + mw, :], pred[:mw, :])
                    del pred_state[m]

            # G_x = x @ W_x.T + bias, precomputed per 128-row aligned group
            gx_tiles = {}

            def emit_gx(g):
                gx = gxp.tile([128, G], F32, tag="gx")
                for n in range(G // NT):
                    ps = gpsum.tile([128, NT], F32, tag="gps")
                    for k in range(KE):
                        nc.tensor.matmul(
                            ps[:, :],
                            XT[:, k, g * 128 : (g + 1) * 128],
                            WX[:, k, n * NT : (n + 1) * NT],
                            start=(k == 0),
                            stop=(k == KE - 1),
                        )
                    nc.vector.tensor_add(
                        gx[:, n * NT : (n + 1) * NT], ps[:, :],
                        BIAS[:, n * NT : (n + 1) * NT],
                    )
                gx_tiles[g] = gx

            emit_gx(0)
            if ngrp > 1:
                emit_gx(1)

            from collections import deque
            pred_units = deque()
            pred_done = 0
            for t in range(Td):
                bt = bs[t]
                g = gidx[t]
                if t > 0 and g != gidx[t - 1] and g + 1 < ngrp:
                    emit_gx(g + 1)
                p0 = gpos[t]
                gx = gx_tiles[g]
                hcol = 0 if t == 0 else B + int(off[t - 1])
                # gate layout (host-permuted): [g | i | f | o]; per-512-column
                # PSUM tiles so elementwise pipelines with the matmul stream
                acts = []
                for n in range(G // NT):
                    ps = gpsum.tile([128, NT], F32, tag="gps")
                    for k in range(KH):
                        nc.tensor.matmul(
                            ps[:, :],
                            HT[:, k, hcol : hcol + 128],
                            WH[:, k, n * NT : (n + 1) * NT],
                            start=(k == 0),
                            stop=(k == KH - 1),
                        )
                    pre = work.tile([B, NT], F32, tag=f"pre{n}")
                    nc.vector.tensor_add(
                        pre[:bt, :], ps[:bt, :],
                        gx[p0 : p0 + bt, n * NT : (n + 1) * NT],
                    )
                    av = work.tile([B, NT], F32, tag=f"act{n}")
                    nc.scalar.activation(
                        av[:bt, :], pre[:bt, :], AF.Tanh if n == 0 else AF.Sigmoid
                    )
                    acts.append(av)
                gv, iv, fv, ov = acts
                ig = work.tile([B, H], F32, tag="ig")
                nc.vector.tensor_mul(ig[:bt, :], iv[:bt, :], gv[:bt, :])
                nc.vector.tensor_mul(CST[:bt, :], fv[:bt, :], CST[:bt, :])
                nc.vector.tensor_add(CST[:bt, :], CST[:bt, :], ig[:bt, :])
                tch = work.tile([B, H], F32, tag="tch")
                nc.scalar.activation(tch[:bt, :], CST[:bt, :], AF.Tanh)
                hbf = work.tile([B, H], BF16, tag="hbf")
                nc.vector.tensor_mul(hbf[:bt, :], ov[:bt, :], tch[:bt, :])
                ocol = B + int(off[t])
                for k in range(KH):
                    tp = tpsum.tile([128, B], BF16, tag="tp")
                    nc.tensor.transpose(
                        tp[:, :bt], hbf[:bt, k * 128 : (k + 1) * 128], IDN[:bt, :bt]
                    )
                    nc.scalar.copy(HT[:, k, ocol : ocol + bt], tp[:, :bt])
                avail = int(off[t]) + bt
                while pred_done < n_chunks and (pred_done + 1) * 128 <= avail:
                    for n in range(VL // PN):
                        pred_units.append((pred_done, n))
                    pred_done += 1
                # spread pred work between steps to keep PE fed during the
                # per-step elementwise chain
                budget = 3 if t < Td - 1 else len(pred_units)
                for _ in range(min(budget, len(pred_units))):
                    emit_pred_unit(*pred_units.popleft())
            while pred_done < n_chunks:
                for n in range(VL // PN):
                    pred_units.append((pred_done, n))
                pred_done += 1
            while pred_units:
                emit_pred_unit(*pred_units.popleft())
    return nc


def _kpack(m):
    """[D, C] (D % 128 == 0) -> [128, D//128, C] K-chunked layout."""
    d, c = m.shape
    return np.ascontiguousarray(
        m.reshape(d // 128, 128, c).transpose(1, 0, 2)
    )


def kernel(image_features, encoded_captions, caption_lengths,
           emb_W, W_ih, W_hh, b_ih, b_hh, lin_W, lin_b):
    global LAST_RESULTS
    bf = ml_dtypes.bfloat16
    img = np.asarray(image_features, np.float32)
    caps_in = np.asarray(encoded_captions)
    cl = np.asarray(caption_lengths)
    embW = np.asarray(emb_W, np.float32)
    Wih = np.asarray(W_ih, np.float32)
    Whh = np.asarray(W_hh, np.float32)
    bih = np.asarray(b_ih, np.float32)
    bhh = np.asarray(b_hh, np.float32)
    linW = np.asarray(lin_W, np.float32)
    linb = np.asarray(lin_b, np.float32)

    lengths = cl[:, 0]
    sort_ind = np.argsort(-lengths, kind="stable")
    caps = caps_in[sort_ind]
    dl = lengths[sort_ind] - 1
    img_s = img[sort_ind]

    Td = int(dl.max())
    bs = [int((dl > t).sum()) for t in range(Td)]
    off = np.concatenate([[0], np.cumsum(bs)]).astype(np.int64)
    na = int(off[Td])

    gidx, gpos, grows = _plan_gx(bs)
    ngrp = len(grows)
    xp = np.zeros((ngrp * 128, E), np.float32)
    for t in range(Td):
        r0 = gidx[t] * 128 + gpos[t]
        xp[r0:r0 + bs[t]] = embW[caps[: bs[t], t]]

    # gate order [g, i, f, o] (torch rows are i, f, g, o)
    perm = np.concatenate([
        np.arange(2 * H, 3 * H), np.arange(0, H),
        np.arange(H, 2 * H), np.arange(3 * H, 4 * H),
    ])
    xt = _kpack(np.ascontiguousarray(xp.T).astype(bf))
    ht0 = _kpack(np.ascontiguousarray(img_s.T).astype(bf))
    wx = _kpack(np.ascontiguousarray(Wih[perm].T).astype(bf))
    wh = _kpack(np.ascontiguousarray(Whh[perm].T).astype(bf))
    bias = np.ascontiguousarray(
        np.broadcast_to((bih + bhh)[perm].astype(np.float32), (128, G))
    )
    has_linb = bool(np.any(linb != 0.0))

    nc = bacc.Bacc("TRN2", target_bir_lowering=False, debug=False,
                   num_devices=NCORES)
    _build(nc, Td, bs, off, na, has_linb)
    nc.finalize()

    in_maps = []
    for c in range(NCORES):
        lw = _kpack(np.ascontiguousarray(linW[c * VL:(c + 1) * VL].T).astype(bf))
        m = dict(xt=xt, ht0=ht0, wx=wx, wh=wh, lw=lw, bias=bias)
        if has_linb:
            m["linb"] = np.ascontiguousarray(
                np.broadcast_to(linb[c * VL:(c + 1) * VL].astype(np.float32),
                                (128, VL)))
        in_maps.append(m)

    res = run_bass_kernel_spmd(
        nc, in_maps, core_ids=list(range(NCORES)),
        trace=TRACE, trace_cores=list(range(NCORES)) if TRACE else None,
        **TRACE_KWARGS,
    )
    LAST_RESULTS = res

    preds = np.zeros((B, TDF, V), np.float32)
    for c in range(NCORES):
        o = res.results[c]["out"]
        sh = slice(c * VL, (c + 1) * VL)
        for t in range(Td):
            preds[: bs[t], t, sh] = o[off[t]:off[t + 1]]
    return preds, caps, dl, sort_ind
